# revision 40
# baseline (speedup 1.0000x reference)
"""Trainium2 Bass kernel for DeepSeek-style attention (B=2, S=2048, H=2048,
NH=16, NKV=4, HD=128, repeat_interleave GQA quirk, RoPE, causal mask).

Sharding: 8 cores = 2 (batch) x 4 (kv-head group).  Each core computes
q/k/v projections for its kv group (4 q heads share 1 kv head), RoPE,
attention, and a partial o_proj against its 512-column slice of Wo.
The 4 partial o_proj outputs per batch are summed on the host.

All layouts are prepared host-side:
  xT   [H, S]        x transposed (contraction dim major), bf16
  wqT  [H, 512]      Wq slice transposed, bf16
  wkT  [H, 128], wvT [H, 128]
  woT  [512, H]      Wo slice transposed (d-major), bf16
  cosT [128, S]      rope cos, head-dim major, bf16
  sinP [128, S]      rope sin, sign-folded + pre-rotated by 64, bf16
  maskb [128, nblk, 128]  unique "mixed" mask blocks, transposed, x sqrt(HD)

Device algorithm highlights:
  * scores are computed transposed ([k, q] layout) so the exp'd probs tile
    is directly the stationary operand of the P@V matmul - no transposes.
  * softmax denominator comes free from a ones-column appended to V
    (contraction over k accumulates sum(exp) in psum column 128).
  * no max-subtraction in softmax (scores are O(5); exp is safe in f32,
    and softmax is shift-invariant so results match the reference).
  * mask blocks are classified host-side: all-zero blocks add nothing,
    blocks entirely < -30 are skipped (exp underflows to 0 relative to
    in-row survivors), mixed blocks get a DVE add of the stored block.
  * DMA runs on two trigger queues: the x/weight stream on sync (SP),
    constants on scalar (Activation); mask/Wo triggers are stalled
    behind the last x chunk so x keeps full HBM bandwidth early.
  * the rotate-by-64 rope permutation matmuls run in bf16 (f32 streams
    at 1/4 rate on the PE) and interleave with the next head's
    projection matmuls so their vector-op latency is hidden.
"""

import math
from contextlib import ExitStack

import ml_dtypes
import numpy as np

import concourse.bass as bass
import concourse.mybir as mybir
import concourse.tile as tile
from concourse import bacc
from concourse.bass_utils import run_bass_kernel_spmd
from concourse.masks import make_identity

B, S, H = 2, 2048, 2048
NH, NKV, HD = 16, 4, 128
P = 128
NB = S // P          # 16 s blocks
HC = H // P          # 16 h chunks
HPG = NH // NKV      # 4 q heads per core
QCH = 512            # q chunk width
NQC = S // QCH       # 4 q chunks
SCALE = 1.0 / math.sqrt(HD)
SQRT_HD = math.sqrt(HD)
F32 = mybir.dt.float32
BF16 = mybir.dt.bfloat16
N_CORES = 8


def _classify_mask(mask):
    """Per 128x128 block: 'zero' (no-op), 'skip' (fully masked), or an index
    into the list of unique transposed/pre-scaled mask blocks."""
    kinds = [[None] * NB for _ in range(NB)]
    uniq, blocks = {}, []
    for qi in range(NB):
        for ki in range(NB):
            sub = mask[qi * P:(qi + 1) * P, ki * P:(ki + 1) * P]
            if not sub.any():
                kinds[qi][ki] = "zero"
            elif sub.max() < -30.0:
                kinds[qi][ki] = "skip"
            else:
                blkT = np.ascontiguousarray(sub.T * SQRT_HD, dtype=np.float32)
                key = blkT.tobytes()
                if key not in uniq:
                    uniq[key] = len(blocks)
                    blocks.append(blkT)
                kinds[qi][ki] = uniq[key]
    return kinds, blocks


def _build_program(kinds, n_blocks):
    nc = bacc.Bacc()
    xT = nc.declare_dram_parameter("xT", [H, S], BF16, isOutput=False)
    wqT = nc.declare_dram_parameter("wqT", [H, HPG * HD], BF16, isOutput=False)
    wkT = nc.declare_dram_parameter("wkT", [H, HD], BF16, isOutput=False)
    wvT = nc.declare_dram_parameter("wvT", [H, HD], BF16, isOutput=False)
    woT = nc.declare_dram_parameter("woT", [HPG * HD, H], BF16, isOutput=False)
    cosT = nc.declare_dram_parameter("cosT", [HD, S], BF16, isOutput=False)
    sinP = nc.declare_dram_parameter("sinP", [HD, S], BF16, isOutput=False)
    perm = nc.declare_dram_parameter("perm", [P, P], BF16, isOutput=False)
    maskb = None
    if n_blocks:
        maskb = nc.declare_dram_parameter("maskb", [P, n_blocks, P], F32,
                                          isOutput=False)
    out = nc.declare_dram_parameter("out", [S, H], BF16, isOutput=True)

    with tile.TileContext(nc) as tc, ExitStack() as ctx:
        consts = ctx.enter_context(tc.tile_pool(name="consts", bufs=1))
        xT_sb = consts.tile([P, HC, S], BF16, tag="xT")
        wqT_sb = consts.tile([P, HC, HPG * HD], BF16, tag="wqT")
        wkT_sb = consts.tile([P, HC, HD], BF16, tag="wkT")
        wvT_sb = consts.tile([P, HC, HD], BF16, tag="wvT")
        woT_sb = consts.tile([P, HPG, H], BF16, tag="woT")
        cos_sb = consts.tile([P, S], BF16, tag="cos")
        sin_sb = consts.tile([P, S], BF16, tag="sin")

        # ONE DMA queue, strict consumption order.  Early HBM bandwidth is
        # the binding constraint (all 8 cores stream their inputs at once,
        # ~250 GB/s/core aggregate): any second queue running concurrently
        # just slows the x chunks that pace the k/v projections.  In-queue
        # ordering is hardware-enforced, so per-chunk {wk, wv, x} triplets
        # give the first matmul its operands ~4us in, and everything later
        # (wq at ~30us, rope constants at ~37us, mask/Wo for attention at
        # ~110us) arrives just ahead of its consumer.
        perm_sb = consts.tile([P, P], BF16, tag="perm")
        for hc in range(HC):
            nc.sync.dma_start(out=wkT_sb[:, hc, :], in_=wkT[hc * P:(hc + 1) * P, :])
            nc.sync.dma_start(out=wvT_sb[:, hc, :], in_=wvT[hc * P:(hc + 1) * P, :])
            if hc < 6:
                # early x chunks in quarters: HBM is contended while all 8
                # cores stream their inputs, so the k/v projections track
                # the DMA at 128KB granularity instead of stalling on
                # whole 512KB chunks.
                for sq in range(NQC):
                    nc.sync.dma_start(
                        out=xT_sb[:, hc, sq * QCH:(sq + 1) * QCH],
                        in_=xT[hc * P:(hc + 1) * P, sq * QCH:(sq + 1) * QCH])
            else:
                nc.sync.dma_start(out=xT_sb[:, hc, :],
                                  in_=xT[hc * P:(hc + 1) * P, :])
        nc.sync.dma_start(out=wqT_sb[:],
                          in_=wqT.rearrange("(c p) f -> p c f", p=P))
        nc.sync.dma_start(out=perm_sb[:], in_=perm[:])
        nc.sync.dma_start(out=cos_sb[:, 0:S // 2], in_=cosT[:, 0:S // 2])
        nc.sync.dma_start(out=sin_sb[:, 0:S // 2], in_=sinP[:, 0:S // 2])
        nc.sync.dma_start(out=cos_sb[:, S // 2:S], in_=cosT[:, S // 2:S])
        nc.sync.dma_start(out=sin_sb[:, S // 2:S], in_=sinP[:, S // 2:S])
        mask_sb = None
        if n_blocks:
            mask_sb = consts.tile([P, n_blocks, P], F32, tag="maskb")
            nc.sync.dma_start(out=mask_sb[:], in_=maskb[:])
        for g in range(HPG):
            nc.sync.dma_start(out=woT_sb[:, g, :], in_=woT[g * P:(g + 1) * P, :])

        # persistent activation buffers
        qrot_sb = consts.tile([P, HPG, S], BF16, tag="qrot")
        krot_sb = consts.tile([P, S], BF16, tag="krot")
        vaug_sb = consts.tile([P, NB, HD + 1], BF16, tag="vaug")
        ident = consts.tile([P, P], BF16, tag="ident")
        make_identity(nc, ident)

        rope_tmp = ctx.enter_context(tc.tile_pool(name="rope_tmp", bufs=4))

        def rope_perms(t1s, us, dsts, us_pool, us_tag, sqs):
            """Finish rope quarters: rot64 via PE perm matmul, then the
            cos/sin recombine add.  dst = t1 + rot64(u), cast to bf16.
            u is bf16 so the perm matmul streams at 1 cycle/row (f32
            pays 4x on the PE)."""
            for sq in sqs:
                us_ps = us_pool.tile([P, QCH], F32, tag=us_tag)
                nc.tensor.matmul(us_ps[:], perm_sb[:], us[sq][:],
                                 start=True, stop=True)
                nc.vector.tensor_add(dsts[sq], t1s[sq][:], us_ps[:])

        with tc.tile_pool(name="proj_ps", bufs=8, space="PSUM") as proj_ps:

            def rope_muls(ps_list):
                """First half of rope: t1 = ps*cos, u = ps*sinP.  All 8
                muls are emitted before any perm so the psum accumulators
                free early and the perm matmuls never serialize behind
                the per-quarter add chain."""
                t1s, us = [], []
                for sq in range(NQC):
                    sl = slice(sq * QCH, (sq + 1) * QCH)
                    t1 = rope_tmp.tile([P, QCH], F32, tag="t1")
                    u = rope_tmp.tile([P, QCH], BF16, tag="u")
                    nc.vector.tensor_mul(t1[:], ps_list[sq][:], cos_sb[:, sl])
                    nc.vector.tensor_mul(u[:], ps_list[sq][:], sin_sb[:, sl])
                    t1s.append(t1)
                    us.append(u)
                return t1s, us

            # k + v projections, h-chunk-major: the PE consumes xT chunks in
            # DMA arrival order (no head-of-line blocking on late chunks).
            # k/v outputs are d-major; 8 accumulators = all 8 psum banks.
            vT_sb = consts.tile([P, S], BF16, tag="vT")
            kps = [proj_ps.tile([P, QCH], F32, tag="ps", name=f"kps{i}")
                   for i in range(NQC)]
            vps = [proj_ps.tile([P, QCH], F32, tag="ps", name=f"vps{i}")
                   for i in range(NQC)]
            for hc in range(HC):
                for sq in range(NQC):
                    nc.tensor.matmul(
                        kps[sq][:], wkT_sb[:, hc, :],
                        xT_sb[:, hc, sq * QCH:(sq + 1) * QCH],
                        start=(hc == 0), stop=(hc == HC - 1))
                for sq in range(NQC):
                    nc.tensor.matmul(
                        vps[sq][:], wvT_sb[:, hc, :],
                        xT_sb[:, hc, sq * QCH:(sq + 1) * QCH],
                        start=(hc == 0), stop=(hc == HC - 1))
            # v psum -> bf16 (frees the 4 v banks); PE-transpose each
            # 128-block to the s-major layout PV needs.
            for sq in range(NQC):
                nc.scalar.copy(out=vT_sb[:, sq * QCH:(sq + 1) * QCH],
                               in_=vps[sq][:])
            for si in range(NB):
                vt = proj_ps.tile([P, P], BF16, tag="ps")
                nc.tensor.transpose(vt[:], vT_sb[:, si * P:(si + 1) * P],
                                    ident[:])
                nc.scalar.copy(out=vaug_sb[:, si, 0:HD], in_=vt[:])
                nc.vector.memset(vaug_sb[:, si, HD:HD + 1], 1.0)

            # q projections, with the PREVIOUS head's rope (and k's rope)
            # interleaved after each head's matmuls: the rope vector-op
            # chains drain while the PE streams the next head's projection.
            kdsts = [krot_sb[:, sq * QCH:(sq + 1) * QCH] for sq in range(NQC)]
            pending = [(kps, kdsts)]  # (psum list, dst slices)
            final_rope = None
            for h in range(HPG):
                qps = [proj_ps.tile([P, QCH], F32, tag="ps", name=f"qps{i}")
                       for i in range(NQC)]
                if h == HPG - 1:
                    # last head: sq-outer order so each quarter finishes
                    # (and its psum-freeing stage copy starts) while the
                    # remaining quarters still stream.
                    for sq in range(NQC):
                        for hc in range(HC):
                            nc.tensor.matmul(
                                qps[sq][:], wqT_sb[:, hc, h * HD:(h + 1) * HD],
                                xT_sb[:, hc, sq * QCH:(sq + 1) * QCH],
                                start=(hc == 0), stop=(hc == HC - 1))
                else:
                    for hc in range(HC):
                        for sq in range(NQC):
                            nc.tensor.matmul(
                                qps[sq][:], wqT_sb[:, hc, h * HD:(h + 1) * HD],
                                xT_sb[:, hc, sq * QCH:(sq + 1) * QCH],
                                start=(hc == 0), stop=(hc == HC - 1))
                ps_list, dsts = pending.pop(0)
                t1s, us = rope_muls(ps_list)
                rope_perms(t1s, us, dsts, proj_ps, "ps", range(NQC))
                pending.append(
                    (qps, [qrot_sb[:, h, sq * QCH:(sq + 1) * QCH]
                           for sq in range(NQC)]))
            # the last head's rope runs entirely inside attention Q0 as PE
            # filler (its qrot isn't needed until Q0's last head).  Stage
            # its psum to SBUF via scalar copies: scalar is idle here, so
            # proj_ps releases ~2us after the last matmul instead of
            # waiting out the rope's vector chain - and the attention
            # pools (same psum banks) unblock that much sooner.
            ps_list, dsts = pending.pop(0)
            qstage = consts.tile([P, NQC, QCH], BF16, tag="qstage")
            for sq in range(NQC):
                nc.scalar.copy(out=qstage[:, sq, :], in_=ps_list[sq][:])
            final_rope = (qstage, dsts)

        # attention pools (reuse banks freed by proj_ps).  qk gets 3 banks
        # so the QK matmul burst can run 3 tiles ahead of the ~3x-slower
        # exp that frees them; tp gets 1 (the attnT chain is latency-
        # tolerant - its consumer is the NEXT chunk's o_proj filler).
        qk_ps = ctx.enter_context(tc.tile_pool(name="qk_ps", bufs=3, space="PSUM"))
        pv_ps = ctx.enter_context(tc.tile_pool(name="pv_ps", bufs=2, space="PSUM"))
        probs_pool = ctx.enter_context(tc.tile_pool(name="probs", bufs=24))
        attnT_pool = ctx.enter_context(tc.tile_pool(name="attnT", bufs=2))
        small = ctx.enter_context(tc.tile_pool(name="small", bufs=4))
        outsb_pool = ctx.enter_context(tc.tile_pool(name="outsb", bufs=2))
        tp_ps = ctx.enter_context(tc.tile_pool(name="tp_ps", bufs=1, space="PSUM"))
        o_ps = ctx.enter_context(tc.tile_pool(name="o_ps", bufs=2, space="PSUM"))

        def oproj_start(Q, attnT, l, copy_eng=None):
            # partial o_proj for row-block l of chunk Q, split into per-oc
            # pieces the caller can interleave as PE filler; one batched
            # output DMA per row-block.  copy_eng picks the psum->sbuf
            # copy engine (the tail blocks use scalar so the copies don't
            # queue behind the last attnT chain on vector).
            si = Q * 4 + l
            osb = outsb_pool.tile([P, QCH * 4], BF16, tag="osb", name="osb")

            def emit(ocs):
                for oc in ocs:
                    po = o_ps.tile([P, QCH], F32, tag="po")
                    for hh in range(HPG):
                        nc.tensor.matmul(
                            po[:], attnT[:, hh, l * P:(l + 1) * P],
                            woT_sb[:, hh, oc * QCH:(oc + 1) * QCH],
                            start=(hh == 0), stop=(hh == HPG - 1))
                    if copy_eng is None:
                        nc.vector.tensor_copy(
                            osb[:, oc * QCH:(oc + 1) * QCH], po[:])
                    else:
                        # tail path: scalar copies (vector is busy with the
                        # last attnT chain) + per-oc DMA so the writeback
                        # drains while later ocs still compute.
                        copy_eng.copy(out=osb[:, oc * QCH:(oc + 1) * QCH],
                                      in_=po[:])
                        nc.sync.dma_start(
                            out=out[si * P:(si + 1) * P,
                                    oc * QCH:(oc + 1) * QCH],
                            in_=osb[:, oc * QCH:(oc + 1) * QCH])

            def finish():
                if copy_eng is None:
                    nc.sync.dma_start(out=out[si * P:(si + 1) * P, :],
                                      in_=osb[:])

            return emit, finish

        def o_proj_block(Q, attnT, l, copy_eng=None):
            emit, finish = oproj_start(Q, attnT, l, copy_eng)
            emit(range(4))
            finish()

        prev = None  # (Q, attnT) pending o_proj, pipelined one chunk behind
        last_pv = []  # final chunk's (attnT, l) pending o_proj, lag 1
        for Q in range(NQC):
            attnT = attnT_pool.tile([P, HPG, QCH], BF16, tag="attnT")
            for h in range(HPG):
                # the previous chunk's o_proj row-block h is the PE filler
                # for this head, split into per-oc pieces: some inside the
                # QK burst (QK can only run 3 sc tiles ahead of the
                # ~3x-slower exp that frees them), the rest packed into
                # the PV chain stalls.
                ofill = oproj_start(prev[0], prev[1], h) if prev is not None \
                    else None
                fill_state = [0]

                def fill(upto):
                    if ofill is None:
                        return
                    while fill_state[0] < min(upto, 4):
                        ofill[0]([fill_state[0]])
                        fill_state[0] += 1
                        if fill_state[0] == 4:
                            ofill[1]()

                if ofill is None and final_rope is not None and h < 2:
                    # Q0 has no previous chunk to o_proj; use the deferred
                    # last-head rope quarters as the filler instead.
                    qstage, dsts = final_rope
                    for sq in (2 * h, 2 * h + 1):
                        sl = slice(sq * QCH, (sq + 1) * QCH)
                        t1 = rope_tmp.tile([P, QCH], BF16, tag="t1b")
                        u = rope_tmp.tile([P, QCH], BF16, tag="u")
                        nc.vector.tensor_mul(t1[:], qstage[:, sq, :],
                                             cos_sb[:, sl])
                        nc.vector.tensor_mul(u[:], qstage[:, sq, :],
                                             sin_sb[:, sl])
                        us_ps = qk_ps.tile([P, QCH], F32, tag="sc")
                        nc.tensor.matmul(us_ps[:], perm_sb[:], u[:],
                                         start=True, stop=True)
                        nc.vector.tensor_add(dsts[sq], t1[:], us_ps[:])

                probs = {}

                def emit_pv(l):
                    # PV for row-block l fires as soon as its diagonal
                    # prob tile is exp'd, interleaving into the QK stream
                    # (QK alone outruns exp and stalls on sc buffers).
                    qi = Q * 4 + l
                    kis = [ki for ki in range(NB)
                           if kinds[qi][ki] != "skip" and ki in probs]
                    if not kis:
                        nc.vector.memset(attnT[:, h, l * P:(l + 1) * P], 0.0)
                    else:
                        pv = pv_ps.tile([P, HD + 1], F32, tag="pv")
                        for j, ki in enumerate(kis):
                            nc.tensor.matmul(
                                pv[:], probs[ki][:, l * P:(l + 1) * P],
                                vaug_sb[:, ki, :],
                                start=(j == 0), stop=(j == len(kis) - 1))
                        recip = small.tile([P, 1], F32, tag="recip")
                        nc.vector.reciprocal(recip[:], pv[:, HD:HD + 1])
                        attn = small.tile([P, P], BF16, tag="attn")
                        nc.vector.tensor_scalar_mul(
                            out=attn[:], in0=pv[:, 0:HD], scalar1=recip[:])
                        tp = tp_ps.tile([P, P], BF16, tag="tp")
                        nc.tensor.transpose(tp[:], attn[:], ident[:])
                        nc.vector.tensor_copy(attnT[:, h, l * P:(l + 1) * P],
                                              tp[:])
                        if Q == NQC - 1 and h == HPG - 1:
                            # final chunk: its own o_proj interleaves into
                            # the last head's PV stream, one block behind
                            # so the attnT write has landed.
                            last_pv.append(l)
                            if len(last_pv) >= 2:
                                o_proj_block(Q, attnT, last_pv.pop(0),
                                             copy_eng=nc.scalar)


                nqk = 0
                for ki in range(NB):
                    cols = [l for l in range(4) if kinds[Q * 4 + l][ki] != "skip"]
                    if not cols:
                        continue
                    lo, hi = min(cols) * P, (max(cols) + 1) * P
                    sc = qk_ps.tile([P, QCH], F32, tag="sc")
                    nc.tensor.matmul(
                        sc[:, lo:hi], krot_sb[:, ki * P:(ki + 1) * P],
                        qrot_sb[:, h, Q * QCH + lo:Q * QCH + hi],
                        start=True, stop=True)
                    for l in cols:
                        kind = kinds[Q * 4 + l][ki]
                        if isinstance(kind, int):
                            nc.vector.tensor_add(
                                sc[:, l * P:(l + 1) * P],
                                sc[:, l * P:(l + 1) * P],
                                mask_sb[:, kind, :])
                    pt = probs_pool.tile([P, QCH], BF16, tag="pt")
                    nc.scalar.activation(
                        out=pt[:, lo:hi], in_=sc[:, lo:hi],
                        func=mybir.ActivationFunctionType.Exp, scale=SCALE)
                    probs[ki] = pt
                    nqk += 1
                    if nqk in (4, 7, 10, 13):
                        # one filler piece every ~3 QK tiles: QK alone
                        # outruns exp by ~3x and stalls once its 3-deep
                        # sc-buffer lead is spent.
                        fill(fill_state[0] + 1)
                    if ki >= Q * 4:
                        emit_pv(ki - Q * 4)
                fill(4)
            prev = (Q, attnT)
        for l in last_pv:
            o_proj_block(prev[0], prev[1], l, copy_eng=nc.scalar)

    nc.compile()
    return nc


_PROGRAM_CACHE = {}


def kernel(x, Wq, Wk, Wv, Wo, cos, sin, attention_mask):
    x = np.asarray(x, dtype=np.float32)
    Wq = np.asarray(Wq, dtype=np.float32)
    Wk = np.asarray(Wk, dtype=np.float32)
    Wv = np.asarray(Wv, dtype=np.float32)
    Wo = np.asarray(Wo, dtype=np.float32)
    cos = np.asarray(cos, dtype=np.float32)
    sin = np.asarray(sin, dtype=np.float32)
    mask = np.asarray(attention_mask, dtype=np.float32)[0, 0]

    kinds, blocks = _classify_mask(mask)
    key = (tuple(tuple(str(k) for k in row) for row in kinds), len(blocks))
    if key not in _PROGRAM_CACHE:
        _PROGRAM_CACHE[key] = _build_program(kinds, len(blocks))
    nc = _PROGRAM_CACHE[key]

    bf = ml_dtypes.bfloat16
    cosT = np.ascontiguousarray(cos[0, 0].T).astype(bf)
    sinT = np.ascontiguousarray(sin[0, 0].T).astype(np.float32)
    sinT[0:64] *= -1.0                                   # fold rotate_half sign
    sinP = np.concatenate([sinT[64:], sinT[:64]], axis=0).astype(bf)  # pre-rot 64
    maskb = np.stack(blocks, axis=1) if blocks else None   # [P, nblk, P]
    dd = np.arange(P)
    permM = (dd[:, None] == (dd[None, :] + 64) % P).astype(bf)

    in_maps = []
    for c in range(N_CORES):
        b, g = c // NKV, c % NKV
        d0, d1 = g * HPG * HD, (g + 1) * HPG * HD
        m = {
            "xT": np.ascontiguousarray(x[b].T).astype(bf),
            "wqT": np.ascontiguousarray(Wq[d0:d1].T).astype(bf),
            "wkT": np.ascontiguousarray(Wk[g * HD:(g + 1) * HD].T).astype(bf),
            "wvT": np.ascontiguousarray(Wv[g * HD:(g + 1) * HD].T).astype(bf),
            "woT": np.ascontiguousarray(Wo[:, d0:d1].T).astype(bf),
            "cosT": cosT,
            "sinP": sinP,
            "perm": permM,
        }
        if maskb is not None:
            m["maskb"] = maskb
        in_maps.append(m)

    global _last_in_maps
    _last_in_maps = in_maps
    res = run_bass_kernel_spmd(nc, in_maps, list(range(N_CORES))).results
    out = np.zeros((B, S, H), np.float32)
    for c in range(N_CORES):
        out[c // NKV] += np.asarray(res[c]["out"]).astype(np.float32)
    return out


# revision 41
# speedup vs baseline: 1.0517x; 1.0517x over previous
"""Trainium2 Bass kernel for DeepSeek-style attention (B=2, S=2048, H=2048,
NH=16, NKV=4, HD=128, repeat_interleave GQA quirk, RoPE, causal mask).

Sharding: 8 cores = 2 (batch) x 4 (kv-head group).  Each core computes
q/k/v projections for its kv group (4 q heads share 1 kv head), RoPE,
attention, and a partial o_proj against its 512-column slice of Wo.
The 4 partial o_proj outputs per batch are summed on the host.

All layouts are prepared host-side:
  xT   [H, S]        x transposed (contraction dim major), bf16
  wqT  [H, 512]      Wq slice transposed, bf16
  wkT  [H, 128], wvT [H, 128]
  woT  [512, H]      Wo slice transposed (d-major), bf16
  cosT [128, S]      rope cos, head-dim major, bf16
  sinP [128, S]      rope sin, sign-folded + pre-rotated by 64, bf16
  maskb [128, nblk, 128]  unique "mixed" mask blocks, transposed, x sqrt(HD)

Device algorithm highlights:
  * scores are computed transposed ([k, q] layout) so the exp'd probs tile
    is directly the stationary operand of the P@V matmul - no transposes.
  * softmax denominator comes free from a ones-column appended to V
    (contraction over k accumulates sum(exp) in psum column 128).
  * no max-subtraction in softmax (scores are O(5); exp is safe in f32,
    and softmax is shift-invariant so results match the reference).
  * mask blocks are classified host-side: all-zero blocks add nothing,
    blocks entirely < -30 are skipped (exp underflows to 0 relative to
    in-row survivors), mixed blocks get a DVE add of the stored block.
  * DMA runs on two trigger queues: the x/weight stream on sync (SP),
    constants on scalar (Activation); mask/Wo triggers are stalled
    behind the last x chunk so x keeps full HBM bandwidth early.
  * the rotate-by-64 rope permutation matmuls run in bf16 (f32 streams
    at 1/4 rate on the PE) and interleave with the next head's
    projection matmuls so their vector-op latency is hidden.
"""

import math
from contextlib import ExitStack

import ml_dtypes
import numpy as np

import concourse.bass as bass
import concourse.mybir as mybir
import concourse.tile as tile
from concourse import bacc
from concourse.bass_utils import run_bass_kernel_spmd
from concourse.masks import make_identity

B, S, H = 2, 2048, 2048
NH, NKV, HD = 16, 4, 128
P = 128
NB = S // P          # 16 s blocks
HC = H // P          # 16 h chunks
HPG = NH // NKV      # 4 q heads per core
QCH = 512            # q chunk width
NQC = S // QCH       # 4 q chunks
SCALE = 1.0 / math.sqrt(HD)
SQRT_HD = math.sqrt(HD)
F32 = mybir.dt.float32
BF16 = mybir.dt.bfloat16
N_CORES = 8


def _classify_mask(mask):
    """Per 128x128 block: 'zero' (no-op), 'skip' (fully masked), or an index
    into the list of unique transposed/pre-scaled mask blocks."""
    kinds = [[None] * NB for _ in range(NB)]
    uniq, blocks = {}, []
    for qi in range(NB):
        for ki in range(NB):
            sub = mask[qi * P:(qi + 1) * P, ki * P:(ki + 1) * P]
            if not sub.any():
                kinds[qi][ki] = "zero"
            elif sub.max() < -30.0:
                kinds[qi][ki] = "skip"
            else:
                blkT = np.ascontiguousarray(sub.T * SQRT_HD, dtype=np.float32)
                key = blkT.tobytes()
                if key not in uniq:
                    uniq[key] = len(blocks)
                    blocks.append(blkT)
                kinds[qi][ki] = uniq[key]
    return kinds, blocks


def _build_program(kinds, n_blocks):
    nc = bacc.Bacc()
    xT = nc.declare_dram_parameter("xT", [H, S], BF16, isOutput=False)
    wqT = nc.declare_dram_parameter("wqT", [H, HPG * HD], BF16, isOutput=False)
    wkT = nc.declare_dram_parameter("wkT", [H, HD], BF16, isOutput=False)
    wvT = nc.declare_dram_parameter("wvT", [H, HD], BF16, isOutput=False)
    woT = nc.declare_dram_parameter("woT", [HPG * HD, H], BF16, isOutput=False)
    cosT = nc.declare_dram_parameter("cosT", [HD, S], BF16, isOutput=False)
    sinP = nc.declare_dram_parameter("sinP", [HD, S], BF16, isOutput=False)
    perm = nc.declare_dram_parameter("perm", [P, P], BF16, isOutput=False)
    maskb = None
    if n_blocks:
        maskb = nc.declare_dram_parameter("maskb", [P, n_blocks, P], F32,
                                          isOutput=False)
    out = nc.declare_dram_parameter("out", [S, H], BF16, isOutput=True)

    with tile.TileContext(nc) as tc, ExitStack() as ctx:
        consts = ctx.enter_context(tc.tile_pool(name="consts", bufs=1))
        xT_sb = consts.tile([P, HC, S], BF16, tag="xT")
        wqT_sb = consts.tile([P, HC, HPG * HD], BF16, tag="wqT")
        wkT_sb = consts.tile([P, HC, HD], BF16, tag="wkT")
        wvT_sb = consts.tile([P, HC, HD], BF16, tag="wvT")
        woT_sb = consts.tile([P, HPG, H], BF16, tag="woT")
        cos_sb = consts.tile([P, S], BF16, tag="cos")
        sin_sb = consts.tile([P, S], BF16, tag="sin")

        # ONE DMA queue, strict consumption order.  Early HBM bandwidth is
        # the binding constraint (all 8 cores stream their inputs at once,
        # ~250 GB/s/core aggregate): any second queue running concurrently
        # just slows the x chunks that pace the k/v projections.  In-queue
        # ordering is hardware-enforced, so per-chunk {wk, wv, x} triplets
        # give the first matmul its operands ~4us in, and everything later
        # (wq at ~30us, rope constants at ~37us, mask/Wo for attention at
        # ~110us) arrives just ahead of its consumer.
        perm_sb = consts.tile([P, P], BF16, tag="perm")
        for hc in range(HC):
            nc.sync.dma_start(out=wkT_sb[:, hc, :], in_=wkT[hc * P:(hc + 1) * P, :])
            nc.sync.dma_start(out=wvT_sb[:, hc, :], in_=wvT[hc * P:(hc + 1) * P, :])
            if hc == 0:
                # first x chunk in quarters: the first k-proj matmul only
                # reads columns 0:512, so it can start ~1.5us sooner.
                for sq in range(NQC):
                    nc.sync.dma_start(
                        out=xT_sb[:, hc, sq * QCH:(sq + 1) * QCH],
                        in_=xT[hc * P:(hc + 1) * P, sq * QCH:(sq + 1) * QCH])
            else:
                nc.sync.dma_start(out=xT_sb[:, hc, :],
                                  in_=xT[hc * P:(hc + 1) * P, :])
        nc.sync.dma_start(out=wqT_sb[:],
                          in_=wqT.rearrange("(c p) f -> p c f", p=P))
        nc.sync.dma_start(out=perm_sb[:], in_=perm[:])
        nc.sync.dma_start(out=cos_sb[:, 0:S // 2], in_=cosT[:, 0:S // 2])
        nc.sync.dma_start(out=sin_sb[:, 0:S // 2], in_=sinP[:, 0:S // 2])
        nc.sync.dma_start(out=cos_sb[:, S // 2:S], in_=cosT[:, S // 2:S])
        nc.sync.dma_start(out=sin_sb[:, S // 2:S], in_=sinP[:, S // 2:S])
        mask_sb = None
        if n_blocks:
            mask_sb = consts.tile([P, n_blocks, P], F32, tag="maskb")
            nc.sync.dma_start(out=mask_sb[:], in_=maskb[:])
        for g in range(HPG):
            nc.sync.dma_start(out=woT_sb[:, g, :], in_=woT[g * P:(g + 1) * P, :])

        # persistent activation buffers
        qrot_sb = consts.tile([P, HPG, S], BF16, tag="qrot")
        krot_sb = consts.tile([P, S], BF16, tag="krot")
        vaug_sb = consts.tile([P, NB, HD + 1], BF16, tag="vaug")
        ident = consts.tile([P, P], BF16, tag="ident")
        make_identity(nc, ident)

        rope_tmp = ctx.enter_context(tc.tile_pool(name="rope_tmp", bufs=4))

        def rope_perms(t1s, us, dsts, us_pool, us_tag, sqs):
            """Finish rope quarters: rot64 via PE perm matmul, then the
            cos/sin recombine add.  dst = t1 + rot64(u), cast to bf16.
            u is bf16 so the perm matmul streams at 1 cycle/row (f32
            pays 4x on the PE)."""
            for sq in sqs:
                us_ps = us_pool.tile([P, QCH], F32, tag=us_tag)
                nc.tensor.matmul(us_ps[:], perm_sb[:], us[sq][:],
                                 start=True, stop=True)
                nc.vector.tensor_add(dsts[sq], t1s[sq][:], us_ps[:])

        with tc.tile_pool(name="proj_ps", bufs=8, space="PSUM") as proj_ps:

            def rope_muls(ps_list):
                """First half of rope: t1 = ps*cos, u = ps*sinP.  All 8
                muls are emitted before any perm so the psum accumulators
                free early and the perm matmuls never serialize behind
                the per-quarter add chain."""
                t1s, us = [], []
                for sq in range(NQC):
                    sl = slice(sq * QCH, (sq + 1) * QCH)
                    t1 = rope_tmp.tile([P, QCH], F32, tag="t1")
                    u = rope_tmp.tile([P, QCH], BF16, tag="u")
                    nc.vector.tensor_mul(t1[:], ps_list[sq][:], cos_sb[:, sl])
                    nc.vector.tensor_mul(u[:], ps_list[sq][:], sin_sb[:, sl])
                    t1s.append(t1)
                    us.append(u)
                return t1s, us

            # k + v projections, h-chunk-major: the PE consumes xT chunks in
            # DMA arrival order (no head-of-line blocking on late chunks).
            # k/v outputs are d-major; 8 accumulators = all 8 psum banks.
            vT_sb = consts.tile([P, S], BF16, tag="vT")
            kps = [proj_ps.tile([P, QCH], F32, tag="ps", name=f"kps{i}")
                   for i in range(NQC)]
            vps = [proj_ps.tile([P, QCH], F32, tag="ps", name=f"vps{i}")
                   for i in range(NQC)]
            for hc in range(HC):
                for sq in range(NQC):
                    nc.tensor.matmul(
                        kps[sq][:], wkT_sb[:, hc, :],
                        xT_sb[:, hc, sq * QCH:(sq + 1) * QCH],
                        start=(hc == 0), stop=(hc == HC - 1))
                for sq in range(NQC):
                    nc.tensor.matmul(
                        vps[sq][:], wvT_sb[:, hc, :],
                        xT_sb[:, hc, sq * QCH:(sq + 1) * QCH],
                        start=(hc == 0), stop=(hc == HC - 1))
            # v psum -> bf16 (frees the 4 v banks); PE-transpose each
            # 128-block to the s-major layout PV needs.
            for sq in range(NQC):
                nc.scalar.copy(out=vT_sb[:, sq * QCH:(sq + 1) * QCH],
                               in_=vps[sq][:])
            for si in range(NB):
                vt = proj_ps.tile([P, P], BF16, tag="ps")
                nc.tensor.transpose(vt[:], vT_sb[:, si * P:(si + 1) * P],
                                    ident[:])
                nc.scalar.copy(out=vaug_sb[:, si, 0:HD], in_=vt[:])
                nc.vector.memset(vaug_sb[:, si, HD:HD + 1], 1.0)

            # q projections, with the PREVIOUS head's rope (and k's rope)
            # interleaved after each head's matmuls: the rope vector-op
            # chains drain while the PE streams the next head's projection.
            kdsts = [krot_sb[:, sq * QCH:(sq + 1) * QCH] for sq in range(NQC)]
            pending = [(kps, kdsts)]  # (psum list, dst slices)
            final_rope = None
            for h in range(HPG):
                qps = [proj_ps.tile([P, QCH], F32, tag="ps", name=f"qps{i}")
                       for i in range(NQC)]
                if h == HPG - 1:
                    # last head: sq-outer order so each quarter finishes
                    # (and its psum-freeing stage copy starts) while the
                    # remaining quarters still stream.
                    for sq in range(NQC):
                        for hc in range(HC):
                            nc.tensor.matmul(
                                qps[sq][:], wqT_sb[:, hc, h * HD:(h + 1) * HD],
                                xT_sb[:, hc, sq * QCH:(sq + 1) * QCH],
                                start=(hc == 0), stop=(hc == HC - 1))
                else:
                    for hc in range(HC):
                        for sq in range(NQC):
                            nc.tensor.matmul(
                                qps[sq][:], wqT_sb[:, hc, h * HD:(h + 1) * HD],
                                xT_sb[:, hc, sq * QCH:(sq + 1) * QCH],
                                start=(hc == 0), stop=(hc == HC - 1))
                ps_list, dsts = pending.pop(0)
                t1s, us = rope_muls(ps_list)
                rope_perms(t1s, us, dsts, proj_ps, "ps", range(NQC))
                pending.append(
                    (qps, [qrot_sb[:, h, sq * QCH:(sq + 1) * QCH]
                           for sq in range(NQC)]))
            # the last head's rope runs entirely inside attention Q0 as PE
            # filler (its qrot isn't needed until Q0's last head).  Stage
            # its psum to SBUF via scalar copies: scalar is idle here, so
            # proj_ps releases ~2us after the last matmul instead of
            # waiting out the rope's vector chain - and the attention
            # pools (same psum banks) unblock that much sooner.
            ps_list, dsts = pending.pop(0)
            qstage = consts.tile([P, NQC, QCH], BF16, tag="qstage")
            for sq in range(NQC):
                nc.scalar.copy(out=qstage[:, sq, :], in_=ps_list[sq][:])
            final_rope = (qstage, dsts)

        # attention pools (reuse banks freed by proj_ps).  qk gets 3 banks
        # so the QK matmul burst can run 3 tiles ahead of the ~3x-slower
        # exp that frees them; tp gets 1 (the attnT chain is latency-
        # tolerant - its consumer is the NEXT chunk's o_proj filler).
        qk_ps = ctx.enter_context(tc.tile_pool(name="qk_ps", bufs=3, space="PSUM"))
        pv_ps = ctx.enter_context(tc.tile_pool(name="pv_ps", bufs=2, space="PSUM"))
        probs_pool = ctx.enter_context(tc.tile_pool(name="probs", bufs=24))
        attnT_pool = ctx.enter_context(tc.tile_pool(name="attnT", bufs=2))
        small = ctx.enter_context(tc.tile_pool(name="small", bufs=4))
        outsb_pool = ctx.enter_context(tc.tile_pool(name="outsb", bufs=2))
        tp_ps = ctx.enter_context(tc.tile_pool(name="tp_ps", bufs=1, space="PSUM"))
        o_ps = ctx.enter_context(tc.tile_pool(name="o_ps", bufs=2, space="PSUM"))

        def oproj_start(Q, attnT, l, copy_eng=None):
            # partial o_proj for row-block l of chunk Q, split into per-oc
            # pieces the caller can interleave as PE filler; one batched
            # output DMA per row-block.  copy_eng picks the psum->sbuf
            # copy engine (the tail blocks use scalar so the copies don't
            # queue behind the last attnT chain on vector).
            si = Q * 4 + l
            osb = outsb_pool.tile([P, QCH * 4], BF16, tag="osb", name="osb")

            def emit(ocs):
                for oc in ocs:
                    po = o_ps.tile([P, QCH], F32, tag="po")
                    for hh in range(HPG):
                        nc.tensor.matmul(
                            po[:], attnT[:, hh, l * P:(l + 1) * P],
                            woT_sb[:, hh, oc * QCH:(oc + 1) * QCH],
                            start=(hh == 0), stop=(hh == HPG - 1))
                    if copy_eng is None:
                        nc.vector.tensor_copy(
                            osb[:, oc * QCH:(oc + 1) * QCH], po[:])
                    else:
                        # tail path: scalar copies (vector is busy with the
                        # last attnT chain) + per-oc DMA so the writeback
                        # drains while later ocs still compute.
                        copy_eng.copy(out=osb[:, oc * QCH:(oc + 1) * QCH],
                                      in_=po[:])
                        nc.sync.dma_start(
                            out=out[si * P:(si + 1) * P,
                                    oc * QCH:(oc + 1) * QCH],
                            in_=osb[:, oc * QCH:(oc + 1) * QCH])

            def finish():
                if copy_eng is None:
                    nc.sync.dma_start(out=out[si * P:(si + 1) * P, :],
                                      in_=osb[:])

            return emit, finish

        def o_proj_block(Q, attnT, l, copy_eng=None):
            emit, finish = oproj_start(Q, attnT, l, copy_eng)
            emit(range(4))
            finish()

        prev = None  # (Q, attnT) pending o_proj, pipelined one chunk behind
        last_pv = []  # final chunk's (attnT, l) pending o_proj, lag 1
        for Q in range(NQC):
            attnT = attnT_pool.tile([P, HPG, QCH], BF16, tag="attnT")
            for h in range(HPG):
                # the previous chunk's o_proj row-block h is the PE filler
                # for this head, split into per-oc pieces: some inside the
                # QK burst (QK can only run 3 sc tiles ahead of the
                # ~3x-slower exp that frees them), the rest packed into
                # the PV chain stalls.
                ofill = oproj_start(prev[0], prev[1], h) if prev is not None \
                    else None
                fill_state = [0]

                def fill(upto):
                    if ofill is None:
                        return
                    while fill_state[0] < min(upto, 4):
                        ofill[0]([fill_state[0]])
                        fill_state[0] += 1
                        if fill_state[0] == 4:
                            ofill[1]()

                if ofill is None and final_rope is not None and h < 2:
                    # Q0 has no previous chunk to o_proj; use the deferred
                    # last-head rope quarters as the filler instead.
                    qstage, dsts = final_rope
                    for sq in (2 * h, 2 * h + 1):
                        sl = slice(sq * QCH, (sq + 1) * QCH)
                        t1 = rope_tmp.tile([P, QCH], BF16, tag="t1b")
                        u = rope_tmp.tile([P, QCH], BF16, tag="u")
                        nc.vector.tensor_mul(t1[:], qstage[:, sq, :],
                                             cos_sb[:, sl])
                        nc.vector.tensor_mul(u[:], qstage[:, sq, :],
                                             sin_sb[:, sl])
                        us_ps = qk_ps.tile([P, QCH], F32, tag="sc")
                        nc.tensor.matmul(us_ps[:], perm_sb[:], u[:],
                                         start=True, stop=True)
                        nc.vector.tensor_add(dsts[sq], t1[:], us_ps[:])

                probs = {}

                def emit_pv(l):
                    # PV for row-block l fires as soon as its diagonal
                    # prob tile is exp'd, interleaving into the QK stream
                    # (QK alone outruns exp and stalls on sc buffers).
                    qi = Q * 4 + l
                    kis = [ki for ki in range(NB)
                           if kinds[qi][ki] != "skip" and ki in probs]
                    if not kis:
                        nc.vector.memset(attnT[:, h, l * P:(l + 1) * P], 0.0)
                    else:
                        pv = pv_ps.tile([P, HD + 1], F32, tag="pv")
                        for j, ki in enumerate(kis):
                            nc.tensor.matmul(
                                pv[:], probs[ki][:, l * P:(l + 1) * P],
                                vaug_sb[:, ki, :],
                                start=(j == 0), stop=(j == len(kis) - 1))
                        recip = small.tile([P, 1], F32, tag="recip")
                        nc.vector.reciprocal(recip[:], pv[:, HD:HD + 1])
                        attn = small.tile([P, P], BF16, tag="attn")
                        nc.vector.tensor_scalar_mul(
                            out=attn[:], in0=pv[:, 0:HD], scalar1=recip[:])
                        tp = tp_ps.tile([P, P], BF16, tag="tp")
                        nc.tensor.transpose(tp[:], attn[:], ident[:])
                        nc.vector.tensor_copy(attnT[:, h, l * P:(l + 1) * P],
                                              tp[:])
                        if Q == NQC - 1 and h == HPG - 1:
                            # final chunk: its own o_proj interleaves into
                            # the last head's PV stream, one block behind
                            # so the attnT write has landed.
                            last_pv.append(l)
                            if len(last_pv) >= 2:
                                o_proj_block(Q, attnT, last_pv.pop(0),
                                             copy_eng=nc.scalar)


                nqk = 0
                for ki in range(NB):
                    cols = [l for l in range(4) if kinds[Q * 4 + l][ki] != "skip"]
                    if not cols:
                        continue
                    lo, hi = min(cols) * P, (max(cols) + 1) * P
                    sc = qk_ps.tile([P, QCH], F32, tag="sc")
                    nc.tensor.matmul(
                        sc[:, lo:hi], krot_sb[:, ki * P:(ki + 1) * P],
                        qrot_sb[:, h, Q * QCH + lo:Q * QCH + hi],
                        start=True, stop=True)
                    for l in cols:
                        kind = kinds[Q * 4 + l][ki]
                        if isinstance(kind, int):
                            nc.vector.tensor_add(
                                sc[:, l * P:(l + 1) * P],
                                sc[:, l * P:(l + 1) * P],
                                mask_sb[:, kind, :])
                    pt = probs_pool.tile([P, QCH], BF16, tag="pt")
                    nc.scalar.activation(
                        out=pt[:, lo:hi], in_=sc[:, lo:hi],
                        func=mybir.ActivationFunctionType.Exp, scale=SCALE)
                    probs[ki] = pt
                    nqk += 1
                    if nqk in (4, 7, 10, 13):
                        # one filler piece every ~3 QK tiles: QK alone
                        # outruns exp by ~3x and stalls once its 3-deep
                        # sc-buffer lead is spent.
                        fill(fill_state[0] + 1)
                    if ki >= Q * 4:
                        emit_pv(ki - Q * 4)
                fill(4)
            prev = (Q, attnT)
        for l in last_pv:
            o_proj_block(prev[0], prev[1], l, copy_eng=nc.scalar)

    nc.compile()
    return nc


_PROGRAM_CACHE = {}


def kernel(x, Wq, Wk, Wv, Wo, cos, sin, attention_mask):
    x = np.asarray(x, dtype=np.float32)
    Wq = np.asarray(Wq, dtype=np.float32)
    Wk = np.asarray(Wk, dtype=np.float32)
    Wv = np.asarray(Wv, dtype=np.float32)
    Wo = np.asarray(Wo, dtype=np.float32)
    cos = np.asarray(cos, dtype=np.float32)
    sin = np.asarray(sin, dtype=np.float32)
    mask = np.asarray(attention_mask, dtype=np.float32)[0, 0]

    kinds, blocks = _classify_mask(mask)
    key = (tuple(tuple(str(k) for k in row) for row in kinds), len(blocks))
    if key not in _PROGRAM_CACHE:
        _PROGRAM_CACHE[key] = _build_program(kinds, len(blocks))
    nc = _PROGRAM_CACHE[key]

    bf = ml_dtypes.bfloat16
    cosT = np.ascontiguousarray(cos[0, 0].T).astype(bf)
    sinT = np.ascontiguousarray(sin[0, 0].T).astype(np.float32)
    sinT[0:64] *= -1.0                                   # fold rotate_half sign
    sinP = np.concatenate([sinT[64:], sinT[:64]], axis=0).astype(bf)  # pre-rot 64
    maskb = np.stack(blocks, axis=1) if blocks else None   # [P, nblk, P]
    dd = np.arange(P)
    permM = (dd[:, None] == (dd[None, :] + 64) % P).astype(bf)

    in_maps = []
    for c in range(N_CORES):
        b, g = c // NKV, c % NKV
        d0, d1 = g * HPG * HD, (g + 1) * HPG * HD
        m = {
            "xT": np.ascontiguousarray(x[b].T).astype(bf),
            "wqT": np.ascontiguousarray(Wq[d0:d1].T).astype(bf),
            "wkT": np.ascontiguousarray(Wk[g * HD:(g + 1) * HD].T).astype(bf),
            "wvT": np.ascontiguousarray(Wv[g * HD:(g + 1) * HD].T).astype(bf),
            "woT": np.ascontiguousarray(Wo[:, d0:d1].T).astype(bf),
            "cosT": cosT,
            "sinP": sinP,
            "perm": permM,
        }
        if maskb is not None:
            m["maskb"] = maskb
        in_maps.append(m)

    global _last_in_maps
    _last_in_maps = in_maps
    res = run_bass_kernel_spmd(nc, in_maps, list(range(N_CORES))).results
    out = np.zeros((B, S, H), np.float32)
    for c in range(N_CORES):
        out[c // NKV] += np.asarray(res[c]["out"]).astype(np.float32)
    return out
